# revision 7
# baseline (speedup 1.0000x reference)
"""Trainium2 Bass kernel for nn_OFModelST: FISTA sparse-coding optical-flow model.

Strategy (validated against the reference):
- Every FISTA call in this problem runs the full 100 iterations (the
  convergence norm plateaus ~0.2, three orders of magnitude above the 1e-4
  early-stop threshold), so fixed-iteration execution matches the reference
  exactly and no cross-core convergence collective is needed.
- Data-parallel over the 16 temporal chunks: core c owns chunks {2c, 2c+1}.
  All phases (spatial FISTA on 80 small units, temporal FISTA on [10,47232])
  run fully independently per core.
- The 41-dim code axis is packed 3x into 123 partitions (block-diagonal
  lhsT), tripling PE/DVE lane utilization.
- FISTA y-momentum is folded into per-iteration scales: stored state is
  z_i = (1+tt_{i-1})*x_i, so each iteration is two matmuls (A@z_j and
  m2s*A@z_{j-1}, accumulated in PSUM) plus ONE fused DVE op
  (u,b) -> (v - clamp(v, -lam, lam)) * c_{i+1} with v = u+b  (custom DVE op).
- Final prediction row (dicTP[T] @ codes) is computed on device; the tiny
  decode einsums (~3 MFLOP) and output assembly run on host.
"""

import os
import sys
from contextlib import ExitStack

import numpy as np

for _p in ("/opt/trn_rl_repo", "/root/.axon_site/_ro/trn_rl_repo"):
    if os.path.isdir(_p) and _p not in sys.path:
        sys.path.append(_p)

import concourse.bacc as bacc
import concourse.bass as bass
import concourse.mybir as mybir
import concourse.tile as tile
from concourse.bass_utils import run_bass_kernel_spmd

# ---------------------------------------------------------------- constants
T, PRE = 10, 1
X_FRA, Y_FRA, NCH = 128, 160, 8
WX, WY = X_FRA // NCH, Y_FRA // NCH          # 16, 20
NCHUNKS = X_FRA // WX                        # 8
N_POLES = 10
CLEN = 4 * N_POLES + 1                       # 41
LAMBD, MAX_ITER = 0.01, 100
F = 2 * CLEN * (X_FRA + Y_FRA)               # 23616
NCORES = 8

P3 = 3 * CLEN                                # 123
HU = 14                                      # H units per block (40 real + 2 pad)
VU = 14
HW_ = HU * Y_FRA                             # 2240  (H block width)
VW_ = VU * X_FRA                             # 1792  (V block width)
TFULL = 2 * F                                # 47232 temporal cols per core
TW = TFULL // 3                              # 15744 packed temporal width
THALF = TW // 2                              # 7872

F32 = mybir.dt.float32

# --------------------------------------------------- custom DVE op
import concourse.dve_ops as dve_ops
from concourse.dve_spec import C0, C1, C2, Spec, Src0, Src1, lower, maxx, minn
from concourse.dve_uop import DveOpSpec


def _register_softshrink():
    name = "SOFTSHRINK_ADD_SC_ANT"
    for o in dve_ops.OPS:
        if o.name == name:
            return o
    _v = Src0 + Src1
    spec = Spec(
        body=(_v - minn(maxx(_v, C0), C1)) * C2,
        reference=lambda in0, in1, s0, s1, imm2: (
            lambda vv: (vv - np.minimum(np.maximum(vv, s0), s1)) * imm2
        )(in0.astype(np.float32) + in1.astype(np.float32)),
    )
    row = dve_ops._CUSTOM_DVE_ROW_BASE + len(dve_ops.OPS)
    assert row < 0x20
    shas = {}
    for ver in ("v3", "v4"):
        uops = lower(spec, ver=ver)
        s = DveOpSpec(name=name, opcode=row, uops=uops, rd1_en=True)
        shas[ver] = s.sha(ver)
    op = dve_ops.DveOp(name, spec, subdim=False, uops_sha=shas)
    dve_ops.OPS.append(op)
    dve_ops.CUSTOM_DVE_SPECS[name] = spec
    dve_ops._SUB_OPCODE_FOR_NAME[name] = row
    return op


SHRINK = _register_softshrink()

# --------------------------------------------------- host-side math helpers


def make_dict(Trows, rr, theta):
    i = np.arange(Trows, dtype=np.float32)[:, None]
    sign = np.where(np.mod(i, 2.0) == 0.0, np.float32(1.0), np.float32(-1.0))
    p = rr[None, :] ** i
    c = np.cos(i * theta[None, :])
    s = np.sin(i * theta[None, :])
    W = np.concatenate(
        [np.ones((Trows, 1), rr.dtype), p * c, sign * p * c, p * s, sign * p * s],
        axis=1,
    ).astype(np.float32)
    G = np.linalg.norm(W, axis=0)
    G = np.where(G == 0.0, np.sqrt(np.float32(Trows)), G).astype(np.float32)
    return (W / G).astype(np.float32)


def fista_mats(D):
    DtD = (D.T @ D).astype(np.float32)
    L = np.float32(np.linalg.norm(DtD))
    linv = np.float32(1.0 / L)
    lam = np.float32(LAMBD * linv)
    A = (np.eye(CLEN, dtype=np.float32) - DtD * linv).astype(np.float32)
    return A, lam, linv


def _tseq():
    t = [np.float32(1.0)]
    for _ in range(MAX_ITER + 1):
        t.append(np.float32((1.0 + np.sqrt(np.float32(1.0 + 4.0 * t[-1] * t[-1]))) / 2.0))
    return [np.float32((t[i] - 1.0) / t[i + 1]) for i in range(MAX_ITER + 1)]


_TT = _tseq()
# stored z_i = c_i * x_i with c_i = 1+tt_{i-1}; c_100 forced to 1 so z_100 = x_100
CS = [np.float32(1.0)] * (MAX_ITER + 1)
for _i in range(1, MAX_ITER):
    CS[_i] = np.float32(1.0 + _TT[_i - 1])
CS[MAX_ITER] = np.float32(1.0)
M2S = [np.float32(0.0)] * MAX_ITER           # scale on A@z_{j-1} at iter i
for _i in range(2, MAX_ITER):
    M2S[_i] = np.float32(-_TT[_i - 1] / (np.float32(1.0) + _TT[_i - 2]))


def diag3(M):
    r, c = M.shape
    out = np.zeros((3 * r, 3 * c), np.float32)
    for b in range(3):
        out[b * r:(b + 1) * r, b * c:(b + 1) * c] = M
    return out


# --------------------------------------------------- device program

# fcol layout per chunk: [H(6560) | V(5248) | HI(6560) | VI(5248)]
KOFF = {"H": 0, "V": CLEN * Y_FRA, "HI": CLEN * (Y_FRA + X_FRA), "VI": CLEN * (2 * Y_FRA + X_FRA)}
KW = {"H": Y_FRA, "V": X_FRA, "HI": Y_FRA, "VI": X_FRA}


def _chunks(total, width):
    out = []
    j = 0
    while j < total:
        w = min(width, total - j)
        out.append((j, w))
        j += w
    return out


def _emit_iters(nc, psp, a2p, streams, niter=MAX_ITER):
    """Emit the 100 scale-folded FISTA iterations for the given streams.

    streams: list of dicts with keys a(SBUF lhsT tile), lam(float),
    za, zb ([123,W] state tiles), b ([123,W]), width.
    """
    cur = {id(s): s["za"] for s in streams}
    prev = {id(s): s["zb"] for s in streams}
    for i in range(niter):
        have_mm2 = i >= 2 and M2S[i] != 0.0
        a2 = {}
        if have_mm2:
            for s in streams:
                t = a2p.tile([P3, P3], F32, name=s["a2tag"], tag=s["a2tag"])
                nc.scalar.mul(t, s["a"], float(M2S[i]))
                a2[id(s)] = t
        imm2 = float(CS[i + 1]) if i + 1 < MAX_ITER else 1.0
        if i == niter - 1:
            imm2 = 1.0
        for s in streams:
            zc, zp = cur[id(s)], prev[id(s)]
            for (j, w) in _chunks(s["width"], 1024):
                u = psp.tile([P3, 1024], F32, tag="u")
                for (sj, sw) in _chunks(w, 512):
                    nc.tensor.matmul(
                        u[:, sj:sj + sw], s["a"], zc[:, j + sj:j + sj + sw],
                        start=True, stop=not have_mm2,
                    )
                    if have_mm2:
                        nc.tensor.matmul(
                            u[:, sj:sj + sw], a2[id(s)], zp[:, j + sj:j + sj + sw],
                            start=False, stop=True,
                        )
                nc.vector._custom_dve(
                    SHRINK, out=zp[:, j:j + w], in0=u[:, 0:w], in1=s["b"][:, j:j + w],
                    s0=-s["lam"], s1=s["lam"], imm2=imm2,
                )
            cur[id(s)], prev[id(s)] = prev[id(s)], cur[id(s)]
    return cur


NITER_OVERRIDE = int(os.environ.get("KERNEL_NITER", str(MAX_ITER)))


def build_nc(lamH, lamV, lamT):
    nc = bacc.Bacc(None, target_bir_lowering=False)

    yh_in = nc.dram_tensor("yh", [3 * WX, HW_], F32, kind="ExternalInput")
    yv_in = nc.dram_tensor("yv", [3 * WY, VW_], F32, kind="ExternalInput")
    ah_in = nc.dram_tensor("ah", [P3, P3], F32, kind="ExternalInput")
    av_in = nc.dram_tensor("av", [P3, P3], F32, kind="ExternalInput")
    at_in = nc.dram_tensor("at", [P3, P3], F32, kind="ExternalInput")
    dh_in = nc.dram_tensor("dh", [3 * WX, P3], F32, kind="ExternalInput")
    dv_in = nc.dram_tensor("dv", [3 * WY, P3], F32, kind="ExternalInput")
    dt_in = nc.dram_tensor("dt", [3 * T, P3], F32, kind="ExternalInput")
    wp_in = nc.dram_tensor("wp", [P3, 3], F32, kind="ExternalInput")
    pred_out = nc.dram_tensor("pred", [3, TW], F32, kind="ExternalOutput")

    with tile.TileContext(nc) as tc:
        with ExitStack() as ctx:
            constp = ctx.enter_context(tc.tile_pool(name="const", bufs=1))
            psp = ctx.enter_context(tc.tile_pool(name="psum", bufs=4, space="PSUM"))
            dramp = ctx.enter_context(tc.tile_pool(name="dram", bufs=1, space="DRAM"))

            ah_t = constp.tile([P3, P3], F32)
            av_t = constp.tile([P3, P3], F32)
            at_t = constp.tile([P3, P3], F32)
            wp_t = constp.tile([P3, 3], F32)
            nc.sync.dma_start(out=ah_t, in_=ah_in[:, :])
            nc.sync.dma_start(out=av_t, in_=av_in[:, :])
            nc.sync.dma_start(out=at_t, in_=at_in[:, :])
            nc.sync.dma_start(out=wp_t, in_=wp_in[:, :])

            # dram scratch holding spatial codes in temporal-natural layout
            ynat = {
                k: dramp.tile([2, T, CLEN, KW[k]], F32, name=f"ynat_{k}", tag=f"ynat_{k}")
                for k in ("H", "V", "HI", "VI")
            }

            # ---------------- spatial phase ----------------
            with ExitStack() as sctx:
                sp = sctx.enter_context(tc.tile_pool(name="spat", bufs=1))
                a2p = sctx.enter_context(tc.tile_pool(name="a2", bufs=2))
                yh_t = sp.tile([3 * WX, HW_], F32)
                yv_t = sp.tile([3 * WY, VW_], F32)
                dh_t = sp.tile([3 * WX, P3], F32)
                dv_t = sp.tile([3 * WY, P3], F32)
                nc.sync.dma_start(out=yh_t, in_=yh_in[:, :])
                nc.sync.dma_start(out=yv_t, in_=yv_in[:, :])
                nc.sync.dma_start(out=dh_t, in_=dh_in[:, :])
                nc.sync.dma_start(out=dv_t, in_=dv_in[:, :])

                bh_t = sp.tile([P3, HW_], F32)
                bv_t = sp.tile([P3, VW_], F32)
                zha = sp.tile([P3, HW_], F32, tag="zha")
                zhb = sp.tile([P3, HW_], F32, tag="zhb")
                zva = sp.tile([P3, VW_], F32, tag="zva")
                zvb = sp.tile([P3, VW_], F32, tag="zvb")
                for t_ in (zha, zhb, zva, zvb):
                    nc.vector.memset(t_, 0.0)

                # b = linv * D^T Y  (3 blocks at once)
                for (dt_w, y_t, b_t, wdt) in (
                    (dh_t, yh_t, bh_t, HW_),
                    (dv_t, yv_t, bv_t, VW_),
                ):
                    for (j, w) in _chunks(wdt, 1024):
                        u = psp.tile([P3, 1024], F32, tag="u")
                        for (sj, sw) in _chunks(w, 512):
                            nc.tensor.matmul(
                                u[:, sj:sj + sw], dt_w, y_t[:, j + sj:j + sj + sw],
                                start=True, stop=True,
                            )
                        nc.scalar.copy(b_t[:, j:j + w], u[:, 0:w])

                streams = [
                    dict(a=ah_t, lam=float(lamH), za=zha, zb=zhb, b=bh_t, width=HW_, a2tag="a2h"),
                    dict(a=av_t, lam=float(lamV), za=zva, zb=zvb, b=bv_t, width=VW_, a2tag="a2v"),
                ]
                cur = _emit_iters(nc, psp, a2p, streams, niter=NITER_OVERRIDE)
                zh_fin = cur[id(streams[0])]
                zv_fin = cur[id(streams[1])]

                # write spatial codes to ynat (skip the 2 pad units per stream)
                for kindset, zfin, w_, nu in (
                    (("H", "HI"), zh_fin, Y_FRA, HU),
                    (("V", "VI"), zv_fin, X_FRA, VU),
                ):
                    for u_ in range(40):
                        lc, kd, fi = u_ // 20, (u_ // 10) % 2, u_ % 10
                        blk, slot = u_ // nu, u_ % nu
                        kind = kindset[kd]
                        nc.sync.dma_start(
                            out=ynat[kind][lc, fi],
                            in_=zfin[CLEN * blk:CLEN * (blk + 1), w_ * slot:w_ * (slot + 1)],
                        )

            # ---------------- temporal phase (two column halves) ----------------
            with ExitStack() as tctx:
                tp = tctx.enter_context(tc.tile_pool(name="temp", bufs=1))
                a2tp = tctx.enter_context(tc.tile_pool(name="a2t", bufs=2))
                prp = tctx.enter_context(tc.tile_pool(name="prp", bufs=2))
                dt_t = tp.tile([3 * T, P3], F32)
                nc.sync.dma_start(out=dt_t, in_=dt_in[:, :])
                bt_t = tp.tile([P3, THALF], F32, tag="bt")
                za = tp.tile([P3, THALF], F32, tag="za")
                zb = tp.tile([P3, THALF], F32, tag="zb")
                for h in range(2):
                    with ExitStack() as yctx:
                        ytp = yctx.enter_context(tc.tile_pool(name="ytp", bufs=1))
                        yt_t = ytp.tile([3 * T, THALF], F32)
                        # gather yt rows: partition 10*bt+i <- fcols [bt*TW + h*THALF, +THALF)
                        for bt in range(3):
                            f0 = bt * TW + h * THALF
                            q = 0
                            while q < THALF:
                                fc = f0 + q
                                lc = fc // F
                                f = fc % F
                                # find kind containing f
                                for kind in ("VI", "HI", "V", "H"):
                                    if f >= KOFF[kind]:
                                        break
                                fk = f - KOFF[kind]
                                run = min(THALF - q, CLEN * KW[kind] - fk)
                                src = ynat[kind][lc].rearrange("i k m -> i (k m)")
                                nc.sync.dma_start(
                                    out=yt_t[10 * bt:10 * bt + 10, q:q + run],
                                    in_=src[:, fk:fk + run],
                                )
                                q += run
                        for (j, w) in _chunks(THALF, 1024):
                            u = psp.tile([P3, 1024], F32, tag="u")
                            for (sj, sw) in _chunks(w, 512):
                                nc.tensor.matmul(
                                    u[:, sj:sj + sw], dt_t, yt_t[:, j + sj:j + sj + sw],
                                    start=True, stop=True,
                                )
                            nc.scalar.copy(bt_t[:, j:j + w], u[:, 0:w])
                    nc.vector.memset(za, 0.0)
                    nc.vector.memset(zb, 0.0)
                    streams = [
                        dict(a=at_t, lam=float(lamT), za=za, zb=zb, b=bt_t, width=THALF, a2tag="a2t"),
                    ]
                    cur = _emit_iters(nc, psp, a2tp, streams, niter=NITER_OVERRIDE)
                    z_fin = cur[id(streams[0])]
                    # prediction row: wp.T @ z -> [3, THALF]
                    for (j, w) in _chunks(THALF, 1024):
                        u3 = psp.tile([3, 1024], F32, tag="u")
                        for (sj, sw) in _chunks(w, 512):
                            nc.tensor.matmul(
                                u3[:, sj:sj + sw], wp_t, z_fin[:, j + sj:j + sj + sw],
                                start=True, stop=True,
                            )
                        ps = prp.tile([3, 1024], F32, tag="ps")
                        nc.scalar.copy(ps[:, 0:w], u3[:, 0:w])
                        nc.sync.dma_start(
                            out=pred_out[:, h * THALF + j: h * THALF + j + w],
                            in_=ps[:, 0:w],
                        )
    nc.compile()
    return nc


_NC_CACHE = {}
LAST_RESULT = None
LAST_RUN_NS = None


def _get_nc(lamH, lamV, lamT):
    key = (float(lamH), float(lamV), float(lamT))
    if key not in _NC_CACHE:
        _NC_CACHE[key] = build_nc(*key)
    return _NC_CACHE[key]


# --------------------------------------------------- host orchestration


def kernel(x, rrT, thetaT, rrSH, thetaSH, rrSV, thetaSV):
    global LAST_RESULT, LAST_RUN_NS
    x = np.asarray(x, np.float32)
    dicH = make_dict(WX, np.asarray(rrSH, np.float32), np.asarray(thetaSH, np.float32))
    dicV = make_dict(WY, np.asarray(rrSV, np.float32), np.asarray(thetaSV, np.float32))
    dicT = make_dict(T, np.asarray(rrT, np.float32), np.asarray(thetaT, np.float32))
    AH, lamH, linvH = fista_mats(dicH)
    AV, lamV, linvV = fista_mats(dicV)
    AT, lamT, linvT = fista_mats(dicT)
    dicTP = make_dict(T + PRE, np.asarray(rrT, np.float32), np.asarray(thetaT, np.float32))
    wpred = dicTP[T, :]                                   # [41]

    AHd = diag3(AH)
    AVd = diag3(AV)
    ATd = diag3(AT)
    DHd = diag3((dicH * linvH).astype(np.float32))        # [48, 123]
    DVd = diag3((dicV * linvV).astype(np.float32))
    DTd = diag3((dicT * linvT).astype(np.float32))        # [30, 123]
    WPd = diag3(wpred[:, None]).astype(np.float32)        # [123, 3]

    in_maps = []
    for core in range(NCORES):
        YH = np.zeros((3 * WX, HW_), np.float32)
        YV = np.zeros((3 * WY, VW_), np.float32)
        for u_ in range(40):
            lc, kd, fi = u_ // 20, (u_ // 10) % 2, u_ % 10
            g = 2 * core + lc
            b, n = g // NCHUNKS, g % NCHUNKS
            frame = x[b, fi].reshape(X_FRA, Y_FRA)
            blk, slot = u_ // HU, u_ % HU
            if kd == 0:
                hmat = frame[WX * n:WX * (n + 1), :]
            else:
                hmat = frame[::-1][WX * n:WX * (n + 1), :]
            YH[WX * blk:WX * (blk + 1), Y_FRA * slot:Y_FRA * (slot + 1)] = hmat
            ft = frame.T
            if kd == 0:
                vmat = ft[WY * n:WY * (n + 1), :]
            else:
                vmat = ft[::-1][WY * n:WY * (n + 1), :]
            blk, slot = u_ // VU, u_ % VU
            YV[WY * blk:WY * (blk + 1), X_FRA * slot:X_FRA * (slot + 1)] = vmat
        in_maps.append(
            dict(yh=YH, yv=YV, ah=AHd, av=AVd, at=ATd, dh=DHd, dv=DVd, dt=DTd, wp=WPd)
        )

    nc = _get_nc(lamH, lamV, lamT)
    import time as _time
    _t0 = _time.time()
    res = run_bass_kernel_spmd(nc, in_maps, core_ids=list(range(NCORES)))
    LAST_RUN_NS = int((_time.time() - _t0) * 1e9)
    LAST_RESULT = res

    # gather: pred[core] [3, TW] flat = fcol-ordered [2, F]
    cPred = np.zeros((2 * NCHUNKS, F), np.float32)
    for core in range(NCORES):
        p = res.results[core]["pred"].reshape(TFULL).reshape(2, F)
        cPred[2 * core:2 * core + 2] = p

    # host decode (reference lines 107-122)
    o = 0
    cH = cPred[:, o:o + CLEN * Y_FRA].reshape(2 * NCHUNKS, CLEN, Y_FRA); o += CLEN * Y_FRA
    cV = cPred[:, o:o + CLEN * X_FRA].reshape(2 * NCHUNKS, CLEN, X_FRA); o += CLEN * X_FRA
    cHI = cPred[:, o:o + CLEN * Y_FRA].reshape(2 * NCHUNKS, CLEN, Y_FRA); o += CLEN * Y_FRA
    cVI = cPred[:, o:].reshape(2 * NCHUNKS, CLEN, X_FRA)
    dsH = make_dict(WX, np.asarray(rrSH, np.float32), np.asarray(thetaSH, np.float32))
    dsV = make_dict(WY, np.asarray(rrSH, np.float32), np.asarray(thetaSH, np.float32))
    B = x.shape[0]
    outH = np.einsum("wk,bkm->bwm", dsH, cH).astype(np.float32).reshape(B, X_FRA, Y_FRA)
    outV = np.einsum("wk,bkm->bwm", dsV, cV).astype(np.float32).reshape(B, Y_FRA, X_FRA)
    outHI = np.einsum("wk,bkm->bwm", dsH, cHI).astype(np.float32).reshape(B, X_FRA, Y_FRA)
    outVI = np.einsum("wk,bkm->bwm", dsV, cVI).astype(np.float32).reshape(B, Y_FRA, X_FRA)
    rev = slice(None, None, -1)
    return (
        outH,
        np.swapaxes(outV, 1, 2),
        outHI[:, rev, :],
        np.swapaxes(outVI, 1, 2)[:, :, rev],
    )
